# revision 35
# baseline (speedup 1.0000x reference)
"""BiGCN (GCN2Conv x4 + BN + head) distributed Bass kernel for 8 Trainium2
NeuronCores.

Strategy
--------
- Nodes sharded contiguously across 8 cores (VALID = N//8 per core, padded to
  SLOTS = multiple of 128). Weights replicated.
- Per layer, the full node-feature table (bf16, [8*SLOTS, 128]) is assembled
  with an AllGather; each core aggregates its in-edges with dma_gather
  (int16 indices relative to one of 4 table ranges) followed by one-hot
  selection matmuls on the tensor engine that accumulate straight into a
  transposed per-window PSUM tile (G^T @ S = agg^T window).
- GCN2Conv algebra folded on host:  h_pre = agg @ A_l + x0 @ B_l  with
  A_l = (1-a)((1-b)I + b W_l), B_l = a((1-b)I + b W_l).
- BatchNorm in transposed layout: free-dim reduces for sums, tiny AllReduce
  for the global batch stats, fused scale/bias + ReLU on the scalar engine.
- Head (lin1 -> BN -> lin2) in the same transposed layout.
"""
import os
import sys
import hashlib
import time

sys.path.insert(0, "/opt/trn_rl_repo")

import numpy as np
import ml_dtypes

import concourse.bass as bass
import concourse.bacc as bacc
import concourse.mybir as mybir
import concourse.tile as tile
from concourse.masks import make_identity

NCORES = 8
L = 4
ALPHA = 0.1
THETA = 0.5
EPS = 1e-5
P = 128
BW = 4          # windows per gather batch
MMBLK = 512     # free-dim block for the dense matmuls

F32 = mybir.dt.float32
BF16 = mybir.dt.bfloat16
I16 = mybir.dt.int16
DEBUG_DUMP = False
BUILD_LAYERS = L       # how many GCN layers to emit (debug knob)
BUILD_HEAD = True      # emit the lin1/BN2/lin2 head (debug knob)
BUILD_PARTS = 15       # bitmask: 1=aggregation 2=dense 4=bn 8=write_table

_CACHE = {}


# --------------------------------------------------------------------------
# host-side preprocessing
# --------------------------------------------------------------------------

def _preprocess(edge_index, N):
    src = np.asarray(edge_index[0], np.int64)
    dst = np.asarray(edge_index[1], np.int64)
    E = src.shape[0]
    assert N % NCORES == 0
    VALID = N // NCORES
    SLOTS = ((VALID + P - 1) // P) * P
    WIN = SLOTS // P
    TROWS = NCORES * SLOTS
    assert TROWS % 4 == 0
    RNG = TROWS // 4
    assert RNG <= 32767

    core = dst // VALID
    slot = dst - core * VALID
    w = slot >> 7
    prow = slot & 127
    trow = (src // VALID) * SLOTS + (src % VALID)
    rng = trow // RNG
    rel = (trow - rng * RNG).astype(np.int16)

    cid = (core * WIN + w) * 4 + rng
    cnt = np.bincount(cid, minlength=NCORES * WIN * 4).reshape(NCORES, WIN, 4)
    sizes = np.maximum(1, -(-cnt.max(axis=0) // P)).astype(np.int64)  # [WIN,4] chunks

    # global column order: w-major, r-minor
    col_off = np.zeros((WIN, 4), np.int64)
    col_off.ravel()[1:] = np.cumsum(sizes.ravel())[:-1]
    TOT = int(sizes.sum())
    # stream (per range) column offsets: cells (w, r) in w order
    scol_off = np.zeros((WIN, 4), np.int64)
    scol_off[1:, :] = np.cumsum(sizes[:-1, :], axis=0)
    stream_cols = sizes.sum(axis=0)  # [4]

    order = np.argsort(cid, kind="stable")
    starts = np.zeros(NCORES * WIN * 4, np.int64)
    flat_cnt = cnt.ravel()
    starts[1:] = np.cumsum(flat_cnt)[:-1]
    posin = np.arange(E, dtype=np.int64) - starts[cid[order]]

    w_s = w[order]
    r_s = rng[order]
    core_s = core[order]
    p_s = prow[order]
    rel_s = rel[order]
    colg = col_off[w_s, r_s] + posin // P
    part = posin % P
    scol = scol_off[w_s, r_s] + posin // P

    dstv = np.full((NCORES, P, TOT), 255.0, np.float32)
    dstv[core_s, part, colg] = p_s
    dstv = dstv.astype(ml_dtypes.bfloat16)

    idx_streams = []
    for r in range(4):
        LEN = int(stream_cols[r]) * P
        st = np.zeros((NCORES, LEN), np.int16)
        m = r_s == r
        st[core_s[m], (scol[m] - scol_off[0, r]) * P + part[m]] = rel_s[m]
        # wrap: [LEN] -> [16, LEN//16] (idx j at [j%16, j//16]) tiled to 128
        wr = st.reshape(NCORES, LEN // 16, 16).transpose(0, 2, 1)
        wr = np.tile(wr, (1, NCORES, 1)).copy()
        idx_streams.append(wr)

    return dict(
        N=N, E=E, VALID=VALID, SLOTS=SLOTS, WIN=WIN, TROWS=TROWS, RNG=RNG,
        sizes=sizes, col_off=col_off, scol_off=scol_off,
        stream_cols=stream_cols, TOT=TOT,
        dstv=dstv, idx_streams=idx_streams,
    )


# --------------------------------------------------------------------------
# program builder
# --------------------------------------------------------------------------

def _build(meta):
    SLOTS, WIN, TROWS, RNG = meta["SLOTS"], meta["WIN"], meta["TROWS"], meta["RNG"]
    VALID, N = meta["VALID"], meta["N"]
    sizes, col_off, scol_off = meta["sizes"], meta["col_off"], meta["scol_off"]
    stream_cols, TOT = meta["stream_cols"], meta["TOT"]

    nblk = (SLOTS + MMBLK - 1) // MMBLK
    blocks = [(b * MMBLK, min(MMBLK, SLOTS - b * MMBLK)) for b in range(nblk)]
    batches = [list(range(b, min(b + BW, WIN))) for b in range(0, WIN, BW)]

    nc = bacc.Bacc("TRN2", target_bir_lowering=False, debug=False,
                   num_devices=NCORES)
    AF = mybir.ActivationFunctionType
    OP = mybir.AluOpType

    # ---- I/O ----
    t_xs = nc.dram_tensor("xs", [SLOTS, P], F32, kind="ExternalInput")
    t_idx = [nc.dram_tensor(f"idx{r}", [P, int(stream_cols[r]) * 8], I16,
                            kind="ExternalInput") for r in range(4)]
    t_dstv = nc.dram_tensor("dstv", [P, TOT], BF16, kind="ExternalInput")
    t_iota = nc.dram_tensor("iota", [P, P], BF16, kind="ExternalInput")
    t_linw = nc.dram_tensor("lin_w", [P, P], F32, kind="ExternalInput")
    t_linb_c = nc.dram_tensor("lin_b_col", [P, 1], F32, kind="ExternalInput")
    t_linb_r = nc.dram_tensor("lin_b_row", [P, P], F32, kind="ExternalInput")
    t_aba = nc.dram_tensor("ab_a", [L, P, P], F32, kind="ExternalInput")
    t_abb = nc.dram_tensor("ab_b", [L, P, P], BF16, kind="ExternalInput")
    t_bn1g = nc.dram_tensor("bn1_g", [P, 1], F32, kind="ExternalInput")
    t_bn1b = nc.dram_tensor("bn1_b", [P, 1], F32, kind="ExternalInput")
    t_l1w = nc.dram_tensor("lin1_w", [P, 16], F32, kind="ExternalInput")
    t_l1b = nc.dram_tensor("lin1_b", [16, 1], F32, kind="ExternalInput")
    t_bn2g = nc.dram_tensor("bn2_g", [16, 1], F32, kind="ExternalInput")
    t_bn2b = nc.dram_tensor("bn2_b", [16, 1], F32, kind="ExternalInput")
    t_l2w = nc.dram_tensor("lin2_w", [16, 1], BF16, kind="ExternalInput")
    t_l2b = nc.dram_tensor("lin2_b", [1, 1], F32, kind="ExternalInput")
    t_out = nc.dram_tensor("out", [1, SLOTS], F32, kind="ExternalOutput")
    t_dbg = (nc.dram_tensor("dbg", [4, P, SLOTS], F32, kind="ExternalOutput")
             if DEBUG_DUMP else None)

    with tile.TileContext(nc) as tc:
        with tc.tile_pool(name="const", bufs=1) as cpool, \
             tc.tile_pool(name="big", bufs=1) as big, \
             tc.tile_pool(name="gpool", bufs=2) as gpool, \
             tc.tile_pool(name="spool", bufs=2) as spool, \
             tc.tile_pool(name="ipool", bufs=2) as ipool, \
             tc.tile_pool(name="stage", bufs=3) as stpool, \
             tc.tile_pool(name="ps_a", bufs=2, space="PSUM") as ps_a, \
             tc.tile_pool(name="ps_b", bufs=2, space="PSUM") as ps_b, \
             tc.tile_pool(name="dram", bufs=1, space="DRAM") as dram:

            # ---------------- constants ----------------
            ident = cpool.tile([P, P], F32)
            make_identity(nc, ident[:])
            iota = cpool.tile([P, P], BF16)
            nc.sync.dma_start(out=iota[:], in_=t_iota[:])
            linw = cpool.tile([P, P], F32)
            nc.sync.dma_start(out=linw[:], in_=t_linw[:])
            linb_c = cpool.tile([P, 1], F32)
            nc.sync.dma_start(out=linb_c[:], in_=t_linb_c[:])
            linb_r = cpool.tile([P, P], F32)
            nc.sync.dma_start(out=linb_r[:], in_=t_linb_r[:])
            aba = cpool.tile([P, L * P], F32)
            nc.sync.dma_start(out=aba[:].rearrange("k (l m) -> k l m", l=L),
                              in_=t_aba[:].rearrange("l k m -> k l m"))
            abb = cpool.tile([P, L * P], BF16)
            nc.sync.dma_start(out=abb[:].rearrange("k (l m) -> k l m", l=L),
                              in_=t_abb[:].rearrange("l k m -> k l m"))
            bn1g = cpool.tile([P, 1], F32)
            nc.sync.dma_start(out=bn1g[:], in_=t_bn1g[:])
            bn1b = cpool.tile([P, 1], F32)
            nc.sync.dma_start(out=bn1b[:], in_=t_bn1b[:])
            l1w = cpool.tile([P, 16], F32)
            nc.sync.dma_start(out=l1w[:], in_=t_l1w[:])
            l1b = cpool.tile([16, 1], F32)
            nc.sync.dma_start(out=l1b[:], in_=t_l1b[:])
            bn2g = cpool.tile([16, 1], F32)
            nc.sync.dma_start(out=bn2g[:], in_=t_bn2g[:])
            bn2b = cpool.tile([16, 1], F32)
            nc.sync.dma_start(out=bn2b[:], in_=t_bn2b[:])
            l2w = cpool.tile([16, 1], BF16)
            nc.sync.dma_start(out=l2w[:], in_=t_l2w[:])
            l2b = cpool.tile([1, 1], F32)
            nc.sync.dma_start(out=l2b[:], in_=t_l2b[:])
            dstv = cpool.tile([P, TOT], BF16)
            nc.sync.dma_start(out=dstv[:], in_=t_dstv[:])

            # ---------------- persistent SBUF ----------------
            x0T = big.tile([P, SLOTS], BF16, tag="x0T")
            hpreT = big.tile([P, SLOTS], BF16, tag="hpreT")
            s2cols = cpool.tile([P, nblk], F32)
            stat2 = cpool.tile([P, 2], F32)
            statg = cpool.tile([P, 2], F32)
            epst = cpool.tile([P, 1], F32)
            nc.vector.memset(epst[:], EPS)

            # ---------------- DRAM scratch ----------------
            bounce = dram.tile([SLOTS, P], BF16)
            tables = [dram.tile([TROWS, P], BF16, addr_space="Shared",
                                tag=f"table{i}", name=f"table{i}")
                      for i in range(L)]
            ar_in = dram.tile([P, 2], F32)
            rgrp = [list(range(NCORES))]

            def stats_and_norm(srcT, dstT, gt, bt, npart, relu, nb, lid):
                """BatchNorm over free dim of srcT[:npart, :SLOTS] with global
                AllReduce; writes dstT = act(scale*src + shift)."""
                s1 = cpool.tile([P, 1], F32, tag="s1")
                nc.vector.reduce_sum(s1[:npart, :], srcT[:npart, :SLOTS],
                                     axis=mybir.AxisListType.X)
                for b, (b0, bw) in enumerate(blocks):
                    sq = stpool.tile([P, MMBLK], F32, tag="sq")
                    nc.vector.tensor_tensor(out=sq[:npart, :bw],
                                            in0=srcT[:npart, b0:b0 + bw],
                                            in1=srcT[:npart, b0:b0 + bw],
                                            op=OP.mult)
                    nc.vector.reduce_sum(s2cols[:npart, b:b + 1],
                                         sq[:npart, :bw],
                                         axis=mybir.AxisListType.X)
                s2 = cpool.tile([P, 1], F32, tag="s2")
                nc.vector.reduce_sum(s2[:npart, :], s2cols[:npart, :nblk],
                                     axis=mybir.AxisListType.X)
                nc.vector.tensor_copy(out=stat2[:npart, 0:1], in_=s1[:npart, :])
                nc.vector.tensor_copy(out=stat2[:npart, 1:2], in_=s2[:npart, :])
                arin = ar_in
                arout = dram.tile([P, 2], F32, addr_space="Shared",
                                  tag=f"arout{lid}", name=f"arout{lid}")
                nc.gpsimd.dma_start(out=arin[:npart, :], in_=stat2[:npart, :])
                nc.gpsimd.collective_compute(
                    "AllReduce", OP.add, replica_groups=rgrp,
                    ins=[arin[:npart, :]], outs=[arout[:npart, :]])
                nc.sync.dma_start(out=statg[:npart, :], in_=arout[:npart, :])
                invN = 1.0 / float(N)
                mean = cpool.tile([P, 1], F32, tag="mean")
                e2 = cpool.tile([P, 1], F32, tag="e2")
                nc.vector.tensor_scalar(out=mean[:npart, :], in0=statg[:npart, 0:1],
                                        scalar1=invN, scalar2=None, op0=OP.mult)
                nc.vector.tensor_scalar(out=e2[:npart, :], in0=statg[:npart, 1:2],
                                        scalar1=invN, scalar2=None, op0=OP.mult)
                var = cpool.tile([P, 1], F32, tag="var")
                nc.vector.tensor_tensor(out=var[:npart, :], in0=mean[:npart, :],
                                        in1=mean[:npart, :], op=OP.mult)
                nc.vector.tensor_tensor(out=var[:npart, :], in0=e2[:npart, :],
                                        in1=var[:npart, :], op=OP.subtract)
                std = cpool.tile([P, 1], F32, tag="std")
                nc.scalar.activation(out=std[:npart, :], in_=var[:npart, :],
                                     func=AF.Sqrt, bias=epst[:npart, :], scale=1.0)
                inv = cpool.tile([P, 1], F32, tag="inv")
                nc.vector.reciprocal(inv[:npart, :], std[:npart, :])
                scv = cpool.tile([P, 1], F32, tag="scv")
                shv = cpool.tile([P, 1], F32, tag="shv")
                nc.vector.tensor_tensor(out=scv[:npart, :], in0=inv[:npart, :],
                                        in1=gt[:npart, :], op=OP.mult)
                nc.vector.tensor_tensor(out=shv[:npart, :], in0=mean[:npart, :],
                                        in1=scv[:npart, :], op=OP.mult)
                nc.vector.tensor_tensor(out=shv[:npart, :], in0=bt[:npart, :],
                                        in1=shv[:npart, :], op=OP.subtract)
                if relu:
                    nc.scalar.activation(out=dstT[:npart, :SLOTS],
                                         in_=srcT[:npart, :SLOTS], func=AF.Relu,
                                         bias=shv[:npart, :], scale=scv[:npart, :])
                else:
                    nc.vector.tensor_scalar(out=dstT[:npart, :SLOTS],
                                            in0=srcT[:npart, :SLOTS],
                                            scalar1=scv[:npart, :],
                                            scalar2=shv[:npart, :],
                                            op0=OP.mult, op1=OP.add)

            def write_table(hsrc, dt_src, table):
                """Transpose hsrc [P feats, SLOTS] into node-major bf16 rows of
                `bounce`, then AllGather into `table`."""
                for b, (b0, bw) in enumerate(blocks):
                    hst = stpool.tile([P, MMBLK], BF16, tag="hst")
                    nwin = bw // P
                    for k in range(nwin):
                        wv = b0 // P + k
                        pt = ps_a.tile([P, P], dt_src, tag="ptr")
                        nc.tensor.transpose(out=pt[:], in_=hsrc[:, wv * P:(wv + 1) * P],
                                            identity=ident[:])
                        nc.vector.tensor_copy(out=hst[:, k * P:(k + 1) * P], in_=pt[:])
                    # hst is [slot, win, feat]; bounce rows are (win, slot)-major
                    nc.sync.dma_start(
                        out=bounce[b0:b0 + bw, :].rearrange("(k s) f -> s k f", k=nwin),
                        in_=hst[:, :bw].rearrange("s (k f) -> s k f", f=P))
                nc.gpsimd.collective_compute(
                    "AllGather", OP.bypass, replica_groups=rgrp,
                    ins=[bounce[:]], outs=[table[:]])

            # ================= encoder =================
            for b, (b0, bw) in enumerate(blocks):
                xtb = stpool.tile([P, MMBLK], F32, tag="xtb")
                nwin = bw // P
                for k in range(nwin):
                    wv = b0 // P + k
                    xc = stpool.tile([P, P], F32, tag="xc")
                    nc.sync.dma_start(out=xc[:], in_=t_xs[wv * P:(wv + 1) * P, :])
                    pt = ps_a.tile([P, P], F32, tag="ptr")
                    nc.tensor.transpose(out=pt[:], in_=xc[:], identity=ident[:])
                    nc.vector.tensor_copy(out=xtb[:, k * P:(k + 1) * P], in_=pt[:])
                    # node-major x0 -> bounce (bf16)
                    p2 = ps_a.tile([P, P], F32, tag="penc")
                    nc.tensor.matmul(out=p2[:], lhsT=xtb[:, k * P:(k + 1) * P],
                                     rhs=linw[:], start=True, stop=True)
                    hs = stpool.tile([P, P], BF16, tag="hs")
                    nc.vector.tensor_tensor(out=hs[:], in0=p2[:], in1=linb_r[:],
                                            op=OP.add)
                    nc.vector.tensor_scalar(out=hs[:], in0=hs[:], scalar1=0.0,
                                            scalar2=None, op0=OP.max)
                    nc.sync.dma_start(out=bounce[wv * P:(wv + 1) * P, :], in_=hs[:])
                # x0T route
                pm = ps_b.tile([P, MMBLK], F32, tag="pmm")
                nc.tensor.matmul(out=pm[:, :bw], lhsT=linw[:], rhs=xtb[:, :bw],
                                 start=True, stop=True)
                nc.scalar.activation(out=x0T[:, b0:b0 + bw], in_=pm[:, :bw],
                                     func=AF.Relu, bias=linb_c[:], scale=1.0)
            nc.gpsimd.collective_compute(
                "AllGather", OP.bypass, replica_groups=rgrp,
                ins=[bounce[:]], outs=[tables[0][:]])

            # ================= layers =================
            hT = None
            for l in range(BUILD_LAYERS):
                table = tables[l]
                aggT = big.tile([P, SLOTS], F32, tag="aggslot")
                if not (BUILD_PARTS & 1) or (BUILD_PARTS & 16):
                    nc.vector.memset(aggT[:], 0.001)
                for batch in (batches if (BUILD_PARTS & 1) else []):
                    w0, wlast = batch[0], batch[-1]
                    gts = []
                    for r in range(4):
                        seg0 = int(scol_off[w0, r])
                        segn = int(sum(sizes[w, r] for w in batch))
                        it = ipool.tile([P, segn * 8], I16, tag=f"it{r}")
                        nc.sync.dma_start(out=it[:],
                                          in_=t_idx[r][:, seg0 * 8:(seg0 + segn) * 8])
                        g = gpool.tile([P, segn * P], BF16, tag=f"g{r}")
                        nc.gpsimd.dma_gather(
                            g[:].rearrange("p (c f) -> p c f", f=P),
                            table[r * RNG:(r + 1) * RNG, :],
                            it[:], segn * P, segn * P, P,
                            single_packet=False)
                        gts.append((g, seg0))
                    for w in batch:
                        if BUILD_PARTS & 32:
                            continue
                        cb = int(col_off[w, 0])
                        nmm = int(sizes[w, :].sum())
                        S = spool.tile([P, nmm * P], BF16, tag="S")
                        dv3 = dstv[:, cb:cb + nmm].rearrange(
                            "p (c one) -> p c one", one=1).to_broadcast([P, nmm, P])
                        io3 = iota[:].rearrange(
                            "p (o f) -> p o f", o=1).to_broadcast([P, nmm, P])
                        nc.vector.tensor_tensor(
                            out=S[:].rearrange("p (c f) -> p c f", f=P),
                            in0=dv3, in1=io3, op=OP.is_equal)
                        if BUILD_PARTS & 16:
                            continue
                        pw = ps_a.tile([P, P], F32, tag="pwin")
                        j = 0
                        for r in range(4):
                            g, seg0 = gts[r]
                            goff = int(scol_off[w, r]) - seg0
                            for q in range(int(sizes[w, r])):
                                jj = int(col_off[w, r]) - cb + q
                                nc.tensor.matmul(
                                    out=pw[:],
                                    lhsT=g[:, (goff + q) * P:(goff + q + 1) * P],
                                    rhs=S[:, jj * P:(jj + 1) * P],
                                    start=(j == 0), stop=(j == nmm - 1))
                                j += 1
                        nc.vector.tensor_copy(out=aggT[:, w * P:(w + 1) * P],
                                              in_=pw[:])
                if DEBUG_DUMP and l == 0:
                    nc.sync.dma_start(out=t_dbg[0], in_=aggT[:])
                # dense: h_preT = A_l^T agg + B_l^T x0
                if not (BUILD_PARTS & 2):
                    nc.vector.memset(hpreT[:], 0.001)
                for b, (b0, bw) in (enumerate(blocks) if (BUILD_PARTS & 2) else []):
                    pm = ps_b.tile([P, MMBLK], F32, tag="pmm")
                    nc.tensor.matmul(out=pm[:, :bw], lhsT=aba[:, l * P:(l + 1) * P],
                                     rhs=aggT[:, b0:b0 + bw], start=True, stop=False)
                    nc.tensor.matmul(out=pm[:, :bw], lhsT=abb[:, l * P:(l + 1) * P],
                                     rhs=x0T[:, b0:b0 + bw], start=False, stop=True)
                    nc.vector.tensor_copy(out=hpreT[:, b0:b0 + bw], in_=pm[:, :bw])
                if VALID < SLOTS:
                    nc.vector.memset(hpreT[:, VALID:SLOTS], 0.0)
                if DEBUG_DUMP and l == 0:
                    nc.gpsimd.dma_start(out=t_dbg[1], in_=hpreT[:])
                hT = big.tile([P, SLOTS], F32, tag="aggslot")
                if BUILD_PARTS & 4:
                    stats_and_norm(hpreT, hT, bn1g, bn1b, P, True, nblk, l)
                else:
                    nc.vector.tensor_copy(out=hT[:, :SLOTS], in_=hpreT[:, :SLOTS])
                if DEBUG_DUMP and l == 0:
                    nc.sync.dma_start(out=t_dbg[2], in_=hT[:])
                    nc.sync.dma_start(out=t_dbg[3][:, 0:2], in_=statg[:])
                if l < L - 1 and (BUILD_PARTS & 8):
                    write_table(hT, F32, tables[l + 1])

            # ================= head =================
            if not BUILD_HEAD:
                src_dbg = hT if hT is not None else x0T
                nc.gpsimd.dma_start(out=t_out[:], in_=src_dbg[0:1, :SLOTS])
            else:
                zT = big.tile([P, SLOTS], BF16, tag="hpreT")
                for b, (b0, bw) in enumerate(blocks):
                    pz = ps_b.tile([P, MMBLK], F32, tag="pmm")
                    nc.tensor.matmul(out=pz[:16, :bw], lhsT=l1w[:],
                                     rhs=hT[:, b0:b0 + bw], start=True, stop=True)
                    nc.vector.tensor_scalar(out=zT[:16, b0:b0 + bw], in0=pz[:16, :bw],
                                            scalar1=l1b[:], scalar2=None, op0=OP.add)
                if VALID < SLOTS:
                    nc.vector.memset(zT[:16, VALID:SLOTS], 0.0)
                zb = big.tile([P, SLOTS], BF16, tag="x0T")
                stats_and_norm(zT, zb, bn2g, bn2b, 16, False, nblk, L)
                for b, (b0, bw) in enumerate(blocks):
                    po = ps_b.tile([P, MMBLK], F32, tag="pmm")
                    nc.tensor.matmul(out=po[:1, :bw], lhsT=l2w[:],
                                     rhs=zb[:16, b0:b0 + bw], start=True, stop=True)
                    ost = stpool.tile([1, MMBLK], F32, tag="ost")
                    nc.vector.tensor_scalar(out=ost[:, :bw], in0=po[:1, :bw],
                                            scalar1=l2b[:], scalar2=None, op0=OP.add)
                    nc.sync.dma_start(out=t_out[0:1, b0:b0 + bw], in_=ost[:, :bw])

    nc.compile()
    return nc


# --------------------------------------------------------------------------
# execution via PJRT (axon)
# --------------------------------------------------------------------------

def _make_runner(nc):
    import jax
    from jax.sharding import Mesh, PartitionSpec
    from jax.experimental.shard_map import shard_map
    from concourse import bass2jax

    bass2jax.install_neuronx_cc_hook()

    partition_name = (nc.partition_id_tensor.name
                      if nc.partition_id_tensor else None)
    in_names, out_names, out_avals, zero_outs = [], [], [], []
    for alloc in nc.m.functions[0].allocations:
        if not isinstance(alloc, mybir.MemoryLocationSet):
            continue
        name = alloc.memorylocations[0].name
        if alloc.kind == "ExternalInput":
            if name != partition_name:
                in_names.append(name)
        elif alloc.kind == "ExternalOutput":
            out_names.append(name)
            shape = tuple(alloc.tensor_shape)
            dtype = mybir.dt.np(alloc.dtype)
            out_avals.append(jax.core.ShapedArray(shape, dtype))
            zero_outs.append(np.zeros(shape, dtype))
    n_params = len(in_names)
    all_names = in_names + out_names
    if partition_name is not None:
        all_names = all_names + [partition_name]
    donate = tuple(range(n_params, n_params + len(out_names)))

    def _body(*args):
        operands = list(args)
        if partition_name is not None:
            operands.append(bass2jax.partition_id_tensor())
        outs = bass2jax._bass_exec_p.bind(
            *operands, out_avals=tuple(out_avals), in_names=tuple(all_names),
            out_names=tuple(out_names), lowering_input_output_aliases=(),
            sim_require_finite=False, sim_require_nnan=False, nc=nc)
        return tuple(outs)

    devices = jax.devices()[:NCORES]
    mesh = Mesh(np.asarray(devices), ("core",))
    in_specs = (PartitionSpec("core"),) * (n_params + len(out_names))
    out_specs = (PartitionSpec("core"),) * len(out_names)
    sharded = jax.jit(
        shard_map(_body, mesh=mesh, in_specs=in_specs, out_specs=out_specs,
                  check_rep=False),
        donate_argnums=donate, keep_unused=True)
    return dict(fn=sharded, in_names=in_names, out_names=out_names,
                out_avals=out_avals, zero_outs=zero_outs, mesh=mesh)


LAST_TIMING_NS = None


def _run(runner, in_maps, timing_iters=0):
    global LAST_TIMING_NS
    import jax
    n_params = len(runner["in_names"])
    concat_in = [
        np.concatenate([np.asarray(in_maps[c][nm]) for c in range(NCORES)], axis=0)
        for nm in runner["in_names"]]
    concat_zeros = [np.zeros((NCORES * z.shape[0], *z.shape[1:]), z.dtype)
                    for z in runner["zero_outs"]]
    if timing_iters:
        from jax.sharding import NamedSharding, PartitionSpec
        sh = NamedSharding(runner["mesh"], PartitionSpec("core"))
        dev_in = [jax.device_put(a, sh) for a in concat_in]
        outs = runner["fn"](*dev_in, *[z.copy() for z in concat_zeros])
        jax.block_until_ready(outs)
        times = []
        for _ in range(timing_iters):
            zs = [z.copy() for z in concat_zeros]
            t0 = time.perf_counter_ns()
            outs = runner["fn"](*dev_in, *zs)
            jax.block_until_ready(outs)
            times.append(time.perf_counter_ns() - t0)
        LAST_TIMING_NS = min(times)
        out_arrs = outs
    else:
        out_arrs = runner["fn"](*concat_in, *concat_zeros)
        jax.block_until_ready(out_arrs)
    res = []
    for c in range(NCORES):
        d = {}
        for i, nm in enumerate(runner["out_names"]):
            shp = runner["out_avals"][i].shape
            d[nm] = np.asarray(out_arrs[i]).reshape(NCORES, *shp)[c]
        res.append(d)
    return res


# --------------------------------------------------------------------------
# public entry
# --------------------------------------------------------------------------

def _fold_weights(conv_w):
    A = np.zeros((L, P, P), np.float32)
    B = np.zeros((L, P, P), np.float32)
    I = np.eye(P, dtype=np.float32)
    for l in range(L):
        beta = float(np.log(THETA / (l + 1) + 1.0))
        Wp = (1.0 - beta) * I + beta * np.asarray(conv_w[l], np.float32)
        A[l] = (1.0 - ALPHA) * Wp
        B[l] = ALPHA * Wp
    return A, B


def kernel(x, edge_index, lin_w, lin_b, conv_w, bn1_g, bn1_b,
           lin1_w, lin1_b, bn2_g, bn2_b, lin2_w, lin2_b,
           _timing_iters=0):
    x = np.asarray(x, np.float32)
    N = x.shape[0]
    ekey = hashlib.sha256(np.ascontiguousarray(edge_index).tobytes()
                          + str(N).encode()).hexdigest()
    if ekey not in _CACHE:
        meta = _preprocess(edge_index, N)
        nc = _build(meta)
        runner = _make_runner(nc)
        _CACHE[ekey] = (meta, runner)
    meta, runner = _CACHE[ekey]
    SLOTS, VALID = meta["SLOTS"], meta["VALID"]

    A, B = _fold_weights(conv_w)
    iota_np = np.tile(np.arange(P, dtype=np.float32),
                      (P, 1)).astype(ml_dtypes.bfloat16)
    lin_b = np.asarray(lin_b, np.float32)
    shared = {
        "iota": iota_np,
        "lin_w": np.asarray(lin_w, np.float32),
        "lin_b_col": lin_b.reshape(P, 1),
        "lin_b_row": np.tile(lin_b.reshape(1, P), (P, 1)),
        "ab_a": A, "ab_b": B.astype(ml_dtypes.bfloat16),
        "bn1_g": np.asarray(bn1_g, np.float32).reshape(P, 1),
        "bn1_b": np.asarray(bn1_b, np.float32).reshape(P, 1),
        "lin1_w": np.asarray(lin1_w, np.float32),
        "lin1_b": np.asarray(lin1_b, np.float32).reshape(16, 1),
        "bn2_g": np.asarray(bn2_g, np.float32).reshape(16, 1),
        "bn2_b": np.asarray(bn2_b, np.float32).reshape(16, 1),
        "lin2_w": np.asarray(lin2_w, np.float32).astype(ml_dtypes.bfloat16),
        "lin2_b": np.asarray(lin2_b, np.float32).reshape(1, 1),
    }
    in_maps = []
    for c in range(NCORES):
        xs = np.zeros((SLOTS, P), np.float32)
        xs[:VALID] = x[c * VALID:(c + 1) * VALID]
        m = {"xs": xs, "dstv": meta["dstv"][c]}
        for r in range(4):
            m[f"idx{r}"] = meta["idx_streams"][r][c]
        m.update(shared)
        in_maps.append(m)

    res = _run(runner, in_maps, timing_iters=_timing_iters)
    out = np.empty((N, 1), np.float32)
    for c in range(NCORES):
        out[c * VALID:(c + 1) * VALID, 0] = res[c]["out"][0, :VALID]
    return out
